# revision 1
# baseline (speedup 1.0000x reference)
"""Trainium2 Bass kernel for CrossAttention (B=2, N=2048, C=768, H=12).

Sharding: core c -> batch b=c//4, head-group g=c%4 (3 heads each).
Each core computes Q/K/V projections for its heads over the full sequence and
attention; an AllToAll exchanges per-head outputs so each core then computes
the full output projection, residual and LayerNorm for its own 512-row
q-shard.

kernel(**inputs) takes the FULL inputs (setup_inputs() keys) and returns the
full [2, 2048, 768] output.
"""

import sys

for _p in ("/opt/trn_rl_repo",):
    if _p not in sys.path:
        sys.path.insert(0, _p)

import numpy as np

B, N, C = 2, 2048, 768
H = 12
DH = 64
EPS = 1e-5
SCALE = DH ** (-0.5)  # 0.125

NCORES = 8
GROUPS = [[0, 1, 2, 3], [4, 5, 6, 7]]
HPC = 3          # heads per core
CS = HPC * DH    # 192 output-feature slice per core
QS = N // 4      # 512 q rows per core
P = 128

_NC_CACHE = {}

# Wo row permutation: gathered AllToAll order is [per-group heads (3g, 3g+1)]
# then [per-group head 3g+2]; Wo rows must match.
import numpy as _np
WO_PERM = _np.concatenate(
    [_np.arange(192 * g, 192 * g + 128) for g in range(4)]
    + [_np.arange(192 * g + 128, 192 * (g + 1)) for g in range(4)]
)


def _build_nc():
    import concourse.bass as bass
    import concourse.mybir as mybir
    import concourse.tile as tile
    from concourse.tile import add_dep_helper
    from concourse import bacc

    f32 = mybir.dt.float32
    bf16 = mybir.dt.bfloat16
    Alu = mybir.AluOpType
    Act = mybir.ActivationFunctionType

    nc = bacc.Bacc(
        "TRN2",
        target_bir_lowering=False,
        debug=False,
        enable_asserts=True,
        num_devices=NCORES,
    )

    # ---- kernel I/O (per-core shapes; host shards the full problem) ----
    qT = nc.dram_tensor("qT", [C, N], bf16, kind="ExternalInput").ap()
    kT = nc.dram_tensor("kT", [C, N], bf16, kind="ExternalInput").ap()
    vT = nc.dram_tensor("vT", [C, N], bf16, kind="ExternalInput").ap()
    wq = nc.dram_tensor("wq", [C, CS], bf16, kind="ExternalInput").ap()
    wk = nc.dram_tensor("wk", [C, CS], bf16, kind="ExternalInput").ap()
    wv = nc.dram_tensor("wv", [C, CS], bf16, kind="ExternalInput").ap()
    wo = nc.dram_tensor("wo", [C, C], bf16, kind="ExternalInput").ap()
    bq = nc.dram_tensor("bq", [CS], f32, kind="ExternalInput").ap()
    bk = nc.dram_tensor("bk", [CS], f32, kind="ExternalInput").ap()
    bv = nc.dram_tensor("bv", [CS], f32, kind="ExternalInput").ap()
    bo = nc.dram_tensor("bo", [C], f32, kind="ExternalInput").ap()
    gamma = nc.dram_tensor("gamma", [C], f32, kind="ExternalInput").ap()
    beta = nc.dram_tensor("beta", [C], f32, kind="ExternalInput").ap()
    qres = nc.dram_tensor("qres", [QS, C], f32, kind="ExternalInput").ap()
    gsel = nc.dram_tensor("gsel", [2], f32, kind="ExternalInput").ap()
    y = nc.dram_tensor("y", [QS, C], f32, kind="ExternalOutput").ap()

    CI = C // P          # 6 contraction chunks
    NJ = N // 512        # 4 n-chunks of 512
    NM = N // P          # 16 kv-chunks of 128
    VS = DH + 1          # 65: v columns + ones column (denominator row)

    with tile.TileContext(nc) as tc:
        const = tc.alloc_tile_pool(name="const", bufs=1)
        persist = tc.alloc_tile_pool(name="persist", bufs=1)
        rows = tc.alloc_tile_pool(name="rows", bufs=2)
        ppool = tc.alloc_tile_pool(name="ppool", bufs=3)
        small = tc.alloc_tile_pool(name="small", bufs=4)
        dram = tc.alloc_tile_pool(name="dram", bufs=1, space="DRAM")

        # ---- constants ----
        wq_sb = const.tile([P, CI, CS], bf16, name="wq_sb")
        wk_sb = const.tile([P, CI, CS], bf16, name="wk_sb")
        wv_sb = const.tile([P, CI, CS], bf16, name="wv_sb")
        nc.sync.dma_start(wk_sb[:], wk.rearrange("(o p) m -> p o m", p=P))
        nc.sync.dma_start(wq_sb[:], wq.rearrange("(o p) m -> p o m", p=P))
        nc.sync.dma_start(wv_sb[:], wv.rearrange("(o p) m -> p o m", p=P))
        wo_sb = const.tile([P, CI, C], bf16, name="wo_sb")

        bqA = const.tile([P, 1], f32, name="bqA")
        bqB = const.tile([DH, 1], f32, name="bqB")
        bkA = const.tile([P, 1], f32, name="bkA")
        bkB = const.tile([DH, 1], f32, name="bkB")
        nc.sync.dma_start(bkA[:], bk[0:P][:, None])
        nc.sync.dma_start(bkB[:], bk[P:CS][:, None])
        nc.sync.dma_start(bqA[:], bq[0:P][:, None])
        nc.sync.dma_start(bqB[:], bq[P:CS][:, None])
        bv_b = const.tile([P, CS], f32, name="bv_b")
        bo_b = const.tile([P, C], f32, name="bo_b")
        gamma_b = const.tile([P, C], f32, name="gamma_b")
        beta_b = const.tile([P, C], f32, name="beta_b")
        nc.sync.dma_start(bv_b[0:1, :], bv[None, :])
        nc.gpsimd.partition_broadcast(bv_b[:], bv_b[0:1, :])
        qres_sb = const.tile([P, QS // P, C], f32, name="qres_sb")

        # ---- persistent activations ----
        qTa = persist.tile([P, N], bf16, name="qTa")    # heads 0,1
        qTb = persist.tile([DH, N], bf16, name="qTb")   # head 2
        kTa = persist.tile([P, N], bf16, name="kTa")
        kTb = persist.tile([DH, N], bf16, name="kTb")
        vaug = persist.tile([P, NM, HPC * VS], bf16, name="vaug")
        nc.vector.memset(
            vaug.rearrange("p m (h d) -> p m h d", d=VS)[:, :, :, DH : DH + 1], 1.0
        )
        gs = const.tile([1, 2], f32, name="gs")
        s0b = const.tile([P, 1], f32, name="s0b")
        s1b = const.tile([P, 1], f32, name="s1b")
        o_h = [persist.tile([DH, N], f32, name=f"o{h}") for h in range(HPC)]
        o_hb = [persist.tile([DH, N], bf16, name=f"ob{h}") for h in range(HPC)]
        l_sb = persist.tile([1, 1024], f32, name="l_sb")
        r_sb = persist.tile([1, 1024], f32, name="r_sb")
        oG = persist.tile([P, CI, QS], bf16, name="oG")

        a2a_in = dram.tile([2 * NJ, CS, QS], bf16, name="a2a_in")
        a2a_out = dram.tile([2 * NJ, CS, QS], bf16, name="a2a_out")

        # ================= Stage A: projections =================
        with tc.tile_pool(name="ppA", bufs=8, space="PSUM") as ppA:
            # --- K ---
            pk_a = [ppA.tile([P, 512], f32, tag="acc", name=f"pka{j}") for j in range(NJ)]
            pk_b = [ppA.tile([P, 512], f32, tag="acc", name=f"pkb{j}") for j in range(NJ)]
            for i in range(CI):
                k_row = rows.tile([P, N], bf16, tag="row", bufs=6, name="k_row")
                nc.sync.dma_start(k_row[:], kT[P * i : P * (i + 1), :])
                st = dict(start=(i == 0), stop=(i == CI - 1))
                for j in range(NJ):
                    s5 = slice(512 * j, 512 * (j + 1))
                    nc.tensor.matmul(pk_a[j][:], wk_sb[:, i, 0:P], k_row[:, s5], **st)
                    nc.tensor.matmul(pk_b[j][0:DH], wk_sb[:, i, P:CS], k_row[:, s5], **st)
            for j in range(NJ):
                s5 = slice(512 * j, 512 * (j + 1))
                nc.vector.tensor_tensor(
                    kTa[:, s5], pk_a[j][:], bkA.to_broadcast((P, 512)), Alu.add
                )
                nc.vector.tensor_tensor(
                    kTb[:, s5], pk_b[j][0:DH], bkB.to_broadcast((DH, 512)), Alu.add
                )
            # --- Q ---
            pq_a = [ppA.tile([P, 512], f32, tag="acc", name=f"pqa{j}") for j in range(NJ)]
            pq_b = [ppA.tile([P, 512], f32, tag="acc", name=f"pqb{j}") for j in range(NJ)]
            for i in range(CI):
                q_row = rows.tile([P, N], bf16, tag="row", bufs=6, name="q_row")
                nc.sync.dma_start(q_row[:], qT[P * i : P * (i + 1), :])
                st = dict(start=(i == 0), stop=(i == CI - 1))
                for j in range(NJ):
                    s5 = slice(512 * j, 512 * (j + 1))
                    nc.tensor.matmul(pq_a[j][:], wq_sb[:, i, 0:P], q_row[:, s5], **st)
                    nc.tensor.matmul(pq_b[j][0:DH], wq_sb[:, i, P:CS], q_row[:, s5], **st)
            for j in range(NJ):
                s5 = slice(512 * j, 512 * (j + 1))
                nc.vector.tensor_tensor(
                    qTa[:, s5], pq_a[j][:], bqA.to_broadcast((P, 512)), Alu.add
                )
                nc.vector.tensor_tensor(
                    qTb[:, s5], pq_b[j][0:DH], bqB.to_broadcast((DH, 512)), Alu.add
                )
            # --- V (natural layout, accumulated per kv-chunk) ---
            for half in range(2):
                pv = [
                    ppA.tile([P, CS], f32, tag="acc", name=f"pv{half}_{m8}")
                    for m8 in range(8)
                ]
                for i in range(CI):
                    v_half = rows.tile([P, 1024], bf16, tag="vrow", bufs=8, name="v_half")
                    nc.sync.dma_start(
                        v_half[:], vT[P * i : P * (i + 1), 1024 * half : 1024 * (half + 1)]
                    )
                    st = dict(start=(i == 0), stop=(i == CI - 1))
                    for m8 in range(8):
                        nc.tensor.matmul(
                            pv[m8][:, 0:CS],
                            v_half[:, P * m8 : P * (m8 + 1)],
                            wv_sb[:, i, :],
                            **st,
                        )
                for m8 in range(8):
                    m = 8 * half + m8
                    dst = vaug.rearrange("p m (h d) -> p m h d", d=VS)[:, m, :, 0:DH]
                    nc.vector.tensor_tensor(
                        dst,
                        pv[m8][:, 0:CS].rearrange("p (h d) -> p h d", d=DH),
                        bv_b.rearrange("p (h d) -> p h d", d=DH),
                        Alu.add,
                    )

        # ================= Stage B: attention (software-pipelined) ========
        with (
            tc.tile_pool(name="ppS", bufs=2, space="PSUM") as ppS,
            tc.tile_pool(name="ppO", bufs=4, space="PSUM") as ppO,
        ):
            def kq_of(h):
                if h < 2:
                    return kTa[DH * h : DH * (h + 1)], qTa[DH * h : DH * (h + 1)]
                return kTb[0:DH], qTb[0:DH]

            def evict_divide(h, qh, po):
                qbase = 1024 * qh
                for q2 in range(2):
                    s5 = slice(qbase + 512 * q2, qbase + 512 * (q2 + 1))
                    nc.vector.tensor_copy(o_h[h][:, s5], po[q2][0:DH])
                    nc.vector.tensor_copy(
                        l_sb[0:1, 512 * q2 : 512 * (q2 + 1)],
                        po[q2][DH : DH + 1],
                    )
                sq = slice(qbase, qbase + 1024)
                nc.vector.reciprocal_approx_fast(out=r_sb[:], in_=l_sb[:])
                rb = ppool.tile([DH, 1024], f32, tag="rb", bufs=2, name="rb")
                nc.gpsimd.partition_broadcast(rb[:], r_sb[0:1, :])
                nc.vector.tensor_tensor(
                    o_hb[h][:, sq], o_h[h][:, sq], rb[:], Alu.mult
                )

            for qh in range(2):
                qbase = 1024 * qh
                # --- heads 0 & 1 jointly: score MMs row-packed (disjoint
                # PE row groups 0-63 / 64-127 run concurrently) ---
                po2 = {
                    h: [
                        ppO.tile([P, 512], f32, tag="o", name=f"po{qh}_{h}_{q2}")
                        for q2 in range(2)
                    ]
                    for h in (0, 1)
                }
                pts = {0: [None] * NM, 1: [None] * NM}
                for m in range(NM):
                    s_t = {
                        h: ppS.tile([P, 1024], f32, tag="s", name=f"ps{qh}{h}{m}")
                        for h in (0, 1)
                    }
                    for q2 in range(2):
                        for h in (0, 1):
                            k_t, q_t = kq_of(h)
                            nc.tensor.matmul(
                                s_t[h][:, 512 * q2 : 512 * (q2 + 1)],
                                k_t[:, P * m : P * (m + 1)],
                                q_t[:, qbase + 512 * q2 : qbase + 512 * (q2 + 1)],
                                start=True,
                                stop=True,
                            )
                    for h in (0, 1):
                        pt = ppool.tile([P, 1024], bf16, tag="p", bufs=5, name="pt")
                        nc.scalar.activation(pt[:], s_t[h][:], Act.Exp, scale=SCALE)
                        pts[h][m] = pt
                    if m >= 1:
                        for h in (0, 1):
                            for q2 in range(2):
                                nc.tensor.matmul(
                                    po2[h][q2][0:VS],
                                    vaug[:, m - 1, VS * h : VS * (h + 1)],
                                    pts[h][m - 1][:, 512 * q2 : 512 * (q2 + 1)],
                                    start=(m - 1 == 0),
                                    stop=False,
                                )
                            pts[h][m - 1] = None
                for h in (0, 1):
                    for q2 in range(2):
                        nc.tensor.matmul(
                            po2[h][q2][0:VS],
                            vaug[:, NM - 1, VS * h : VS * (h + 1)],
                            pts[h][NM - 1][:, 512 * q2 : 512 * (q2 + 1)],
                            start=False,
                            stop=True,
                        )
                for h in (0, 1):
                    evict_divide(h, qh, po2[h])

            # send heads 0,1 slices early (collective itself runs at the end)
            for r in range(NJ):
                for h in (0, 1):
                    for g2 in range(2):
                        nc.sync.dma_start(
                            a2a_in[NJ * g2 + r, DH * h : DH * (h + 1), :],
                            o_hb[h][:, QS * r : QS * (r + 1)],
                        )

            for qh in range(2):
                qbase = 1024 * qh
                # --- head 2 solo ---
                h = 2
                k_t, q_t = kq_of(h)
                po = [
                    ppO.tile([P, 512], f32, tag="o", name=f"po{qh}_{h}_{q2}")
                    for q2 in range(2)
                ]
                pts2 = [None] * NM
                for m in range(NM):
                    ps = ppS.tile([P, 1024], f32, tag="s", name=f"ps{qh}_{h}_{m}")
                    for q2 in range(2):
                        nc.tensor.matmul(
                            ps[:, 512 * q2 : 512 * (q2 + 1)],
                            k_t[:, P * m : P * (m + 1)],
                            q_t[:, qbase + 512 * q2 : qbase + 512 * (q2 + 1)],
                            start=True,
                            stop=True,
                        )
                    pt = ppool.tile([P, 1024], bf16, tag="p", bufs=5, name="pt")
                    nc.scalar.activation(pt[:], ps[:], Act.Exp, scale=SCALE)
                    pts2[m] = pt
                    if m >= 1:
                        for q2 in range(2):
                            nc.tensor.matmul(
                                po[q2][0:VS],
                                vaug[:, m - 1, VS * h : VS * (h + 1)],
                                pts2[m - 1][:, 512 * q2 : 512 * (q2 + 1)],
                                start=(m - 1 == 0),
                                stop=False,
                            )
                        pts2[m - 1] = None
                for q2 in range(2):
                    nc.tensor.matmul(
                        po[q2][0:VS],
                        vaug[:, NM - 1, VS * h : VS * (h + 1)],
                        pts2[NM - 1][:, 512 * q2 : 512 * (q2 + 1)],
                        start=False,
                        stop=True,
                    )
                evict_divide(h, qh, po)

        # deferred tail-only constant loads (emitted late to keep the
        # startup DMA queue clear for stage A input rows)
        nc.sync.dma_start(wo_sb[:], wo.rearrange("(o p) m -> p o m", p=P))
        nc.sync.dma_start(gs[:], gsel[None, :])
        nc.gpsimd.partition_broadcast(s0b[:], gs[0:1, 0:1])
        nc.gpsimd.partition_broadcast(s1b[:], gs[0:1, 1:2])
        nc.sync.dma_start(bo_b[0:1, :], bo[None, :])
        nc.sync.dma_start(gamma_b[0:1, :], gamma[None, :])
        nc.sync.dma_start(beta_b[0:1, :], beta[None, :])
        nc.gpsimd.partition_broadcast(bo_b[:], bo_b[0:1, :])
        nc.gpsimd.partition_broadcast(gamma_b[:], gamma_b[0:1, :])
        nc.gpsimd.partition_broadcast(beta_b[:], beta_b[0:1, :])
        nc.sync.dma_start(qres_sb[:], qres.rearrange("(t p) c -> p t c", p=P))
        nc.vector.tensor_tensor(
            qres_sb[:],
            qres_sb[:],
            bo_b[:, None, :].to_broadcast((P, QS // P, C)),
            Alu.add,
        )

        # ====== Stage C: AllToAll (all heads) =============================
        for r in range(NJ):
            for g2 in range(2):
                nc.sync.dma_start(
                    a2a_in[NJ * g2 + r, 2 * DH : CS, :],
                    o_hb[2][:, QS * r : QS * (r + 1)],
                )
        nc.gpsimd.collective_compute(
            "AllToAll",
            Alu.bypass,
            replica_groups=[list(range(NCORES))],
            ins=[a2a_in.opt()],
            outs=[a2a_out.opt()],
        )
        nc.sync.dma_start(
            oG[:, 0:4, :],
            a2a_out[0:NJ, 0:P, :].rearrange("r s w -> s r w"),
        )
        oGt1 = rows.tile([P, 4, QS], bf16, tag="row", bufs=6, name="oGt1")
        nc.sync.dma_start(
            oGt1[:],
            a2a_out[NJ : 2 * NJ, 0:P, :].rearrange("r s w -> s r w"),
        )
        nc.vector.tensor_scalar(
            oG[:, 0:4, :], oG[:, 0:4, :], s0b[:], None, Alu.mult
        )
        nc.vector.tensor_scalar(oGt1[:], oGt1[:], s1b[:], None, Alu.mult)
        nc.vector.tensor_tensor(oG[:, 0:4, :], oG[:, 0:4, :], oGt1[:], Alu.add)
        for r2 in range(2):
            nc.sync.dma_start(
                oG[:, 4:6, :].rearrange("(r2 s) o w -> r2 s o w", s=DH)[r2],
                a2a_out[0:NJ, 2 * DH : CS, :].rearrange(
                    "(o r2) s w -> r2 s o w", r2=2
                )[r2],
            )
        oGt2 = rows.tile([P, 2, QS], bf16, tag="vrow", bufs=8, name="oGt2")
        for r2 in range(2):
            nc.sync.dma_start(
                oGt2[:].rearrange("(r2 s) o w -> r2 s o w", s=DH)[r2],
                a2a_out[NJ : 2 * NJ, 2 * DH : CS, :].rearrange(
                    "(o r2) s w -> r2 s o w", r2=2
                )[r2],
            )
        nc.vector.tensor_scalar(
            oG[:, 4:6, :], oG[:, 4:6, :], s0b[:], None, Alu.mult
        )
        nc.vector.tensor_scalar(oGt2[:], oGt2[:], s1b[:], None, Alu.mult)
        nc.vector.tensor_tensor(oG[:, 4:6, :], oG[:, 4:6, :], oGt2[:], Alu.add)

        # ======= Stage D: full Wo (token-major) + residual + LayerNorm ====
        with tc.tile_pool(name="ppD", bufs=2, space="PSUM") as ppD:
            for qt in range(QS // P):
                px = ppD.tile([P, C], f32, tag="d", name=f"px{qt}")
                for ci in range(CI):
                    st = dict(start=(ci == 0), stop=(ci == CI - 1))
                    nc.tensor.matmul(
                        px[:, 0:512],
                        oG[:, ci, P * qt : P * (qt + 1)],
                        wo_sb[:, ci, 0:512],
                        **st,
                    )
                    nc.tensor.matmul(
                        px[:, 512:C],
                        oG[:, ci, P * qt : P * (qt + 1)],
                        wo_sb[:, ci, 512:C],
                        **st,
                    )
                x1 = ppool.tile([P, C], f32, tag="x1", bufs=2, name="x1")
                nc.vector.tensor_tensor(x1[:], px[:], qres_sb[:, qt], Alu.add)
                mu = small.tile([P, 1], f32, tag="st", name="mu")
                sq = ppool.tile([P, C], f32, tag="sq", bufs=2, name="sq")
                sqs = small.tile([P, 1], f32, tag="st", name="sqs")
                var = small.tile([P, 1], f32, tag="st", name="var")
                rinv = small.tile([P, 1], f32, tag="st", name="rinv")
                rstd = small.tile([P, 1], f32, tag="st", name="rstd")
                nb = small.tile([P, 1], f32, tag="st", name="nb")
                nc.vector.reduce_sum(mu[:], x1[:], axis=mybir.AxisListType.X)
                nc.vector.tensor_scalar_mul(mu[:], mu[:], 1.0 / C)
                nc.scalar.activation(sq[:], x1[:], Act.Square, accum_out=sqs[:])
                nc.vector.tensor_scalar_mul(sqs[:], sqs[:], 1.0 / C)
                nc.vector.tensor_tensor(var[:], mu[:], mu[:], Alu.mult)
                nc.vector.tensor_tensor(var[:], sqs[:], var[:], Alu.subtract)
                nc.vector.tensor_scalar_add(var[:], var[:], EPS)
                nc.vector.reciprocal(rinv[:], var[:])
                nc.scalar.activation(rstd[:], rinv[:], Act.Sqrt)
                nc.vector.tensor_tensor(nb[:], mu[:], rstd[:], Alu.mult)
                nc.vector.tensor_scalar_mul(nb[:], nb[:], -1.0)
                nc.vector.tensor_scalar(
                    x1[:], x1[:], rstd[:], nb[:], Alu.mult, Alu.add
                )
                nc.vector.tensor_tensor(x1[:], x1[:], gamma_b[:], Alu.mult)
                nc.vector.tensor_tensor(x1[:], x1[:], beta_b[:], Alu.add)
                nc.sync.dma_start(
                    y.rearrange("(t p) c -> p t c", p=P)[:, qt], x1[:]
                )

        for pool in (dram, small, ppool, rows, persist, const):
            pool.release()

    nc.compile()
    return nc


def get_nc():
    if "nc" not in _NC_CACHE:
        _NC_CACHE["nc"] = _build_nc()
    return _NC_CACHE["nc"]


def make_in_maps(inputs):
    import ml_dtypes

    b16 = ml_dtypes.bfloat16
    q = np.asarray(inputs["query"], np.float32)
    k = np.asarray(inputs["key_in"], np.float32)
    v = np.asarray(inputs["value"], np.float32)
    Wq = np.asarray(inputs["Wq"], np.float32)
    Wk = np.asarray(inputs["Wk"], np.float32)
    Wv = np.asarray(inputs["Wv"], np.float32)
    Wo = np.asarray(inputs["Wo"], np.float32)
    bq = np.asarray(inputs["bq"], np.float32)
    bk = np.asarray(inputs["bk"], np.float32)
    bv = np.asarray(inputs["bv"], np.float32)
    bo = np.asarray(inputs["bo"], np.float32)
    gamma = np.asarray(inputs["gamma"], np.float32)
    beta = np.asarray(inputs["beta"], np.float32)

    in_maps = []
    for c in range(NCORES):
        b, g = c // 4, c % 4
        cs = slice(CS * g, CS * (g + 1))
        in_maps.append(
            {
                "qT": np.ascontiguousarray(q[b].T).astype(b16),
                "kT": np.ascontiguousarray(k[b].T).astype(b16),
                "vT": np.ascontiguousarray(v[b].T).astype(b16),
                "wq": np.ascontiguousarray(Wq[:, cs]).astype(b16),
                "wk": np.ascontiguousarray(Wk[:, cs]).astype(b16),
                "wv": np.ascontiguousarray(Wv[:, cs]).astype(b16),
                "wo": Wo[WO_PERM, :].astype(b16),
                "bq": np.ascontiguousarray(bq[cs]),
                "bk": np.ascontiguousarray(bk[cs]),
                "bv": np.ascontiguousarray(bv[cs]),
                "bo": bo.copy(),
                "gamma": gamma.copy(),
                "beta": beta.copy(),
                "qres": np.ascontiguousarray(q[b, QS * g : QS * (g + 1)]),
                "gsel": np.array([1.0 - b, float(b)], np.float32),
            }
        )
    return in_maps


def _install_ntff_shim():
    """Provide antenv.axon_hooks if the image lacks it (needed for trace=True)."""
    try:
        import antenv.axon_hooks  # noqa: F401

        return
    except ImportError:
        pass
    import contextlib
    import ctypes
    import types

    so_path = "/opt/axon/libaxon_pjrt.so"
    state = {"hook": None}

    def set_axon_ntff_profile_hook(h):
        state["hook"] = h

    def get_axon_ntff_profile_hook():
        if state["hook"] is None:
            try:
                lib = ctypes.CDLL(so_path)
            except OSError:
                return None
            if not hasattr(lib, "axon_start_nrt_profile"):
                return None
            lib.axon_start_nrt_profile.argtypes = [
                ctypes.POINTER(ctypes.c_int64),
                ctypes.c_size_t,
            ]
            lib.axon_start_nrt_profile.restype = ctypes.c_int64
            lib.axon_stop_nrt_profile.argtypes = [ctypes.c_char_p]
            lib.axon_stop_nrt_profile.restype = ctypes.c_int64

            @contextlib.contextmanager
            def _hook(output_dir, device_ids):
                import jax

                jax.devices()
                if device_ids:
                    ids = (ctypes.c_int64 * len(device_ids))(*device_ids)
                    rc = lib.axon_start_nrt_profile(ids, len(device_ids))
                else:
                    rc = lib.axon_start_nrt_profile(None, 0)
                if rc != 0:
                    raise RuntimeError(f"axon_start_nrt_profile rc={rc}")
                try:
                    yield
                finally:
                    n = lib.axon_stop_nrt_profile(str(output_dir).encode())
                    print(f"profile: {n} file(s) written to {output_dir}")

            state["hook"] = _hook
        return state["hook"]

    mod = types.ModuleType("antenv.axon_hooks")
    mod.set_axon_ntff_profile_hook = set_axon_ntff_profile_hook
    mod.get_axon_ntff_profile_hook = get_axon_ntff_profile_hook
    import antenv

    antenv.axon_hooks = mod
    sys.modules["antenv.axon_hooks"] = mod


def run(inputs, trace=False, trace_cores=None):
    if trace:
        _install_ntff_shim()
    from concourse.bass_utils import run_bass_kernel_spmd

    nc = get_nc()
    in_maps = make_in_maps(inputs)
    res = run_bass_kernel_spmd(
        nc,
        in_maps,
        list(range(NCORES)),
        trace=trace,
        **({"trace_cores": trace_cores} if trace_cores is not None else {}),
    )
    out = np.empty((B, N, C), np.float32)
    for c in range(NCORES):
        b, g = c // 4, c % 4
        out[b, QS * g : QS * (g + 1)] = res.results[c]["y"]
    return out, res


def kernel(**inputs):
    out, _ = run(inputs, trace=False)
    return out



# revision 3
# speedup vs baseline: 1.1764x; 1.1764x over previous
"""Trainium2 Bass kernel for CrossAttention (B=2, N=2048, C=768, H=12).

Sharding: core c -> batch b=c//4, head-group g=c%4 (3 heads each).
Each core computes Q/K/V projections for its heads over the full sequence and
attention; an AllToAll exchanges per-head outputs so each core then computes
the full output projection, residual and LayerNorm for its own 512-row
q-shard.

v2: every matmul in the kernel runs in the 128x128 tile mode (per-head K is
zero-padded to the full 128 contraction rows; head-2 projection weights are
zero-padded to 128 output cols) so the PE never drains for a mode switch and
the HAM activity monitor sees a fully-busy array.  The AllToAll is split in
two: heads {0,1} exchange overlaps head-2 attention, and the small head-2
exchange overlaps the heads01 part of the output projection.

kernel(**inputs) takes the FULL inputs (setup_inputs() keys) and returns the
full [2, 2048, 768] output.
"""

import sys

for _p in ("/opt/trn_rl_repo",):
    if _p not in sys.path:
        sys.path.insert(0, _p)

import numpy as np

B, N, C = 2, 2048, 768
H = 12
DH = 64
EPS = 1e-5
SCALE = DH ** (-0.5)  # 0.125

NCORES = 8
HPC = 3          # heads per core
CS = HPC * DH    # 192 output-feature slice per core
CSP = 256        # padded wq/wk col count (head2 half padded 64->128)
QS = N // 4      # 512 q rows per core
P = 128

_NC_CACHE = {}

# Wo row permutation: gathered AllToAll order is [per-group heads (3g, 3g+1)]
# then [per-group head 3g+2]; Wo rows must match.
import numpy as _np
WO_PERM = _np.concatenate(
    [_np.arange(192 * g, 192 * g + 128) for g in range(4)]
    + [_np.arange(192 * g + 128, 192 * (g + 1)) for g in range(4)]
)


def _build_nc():
    import concourse.bass as bass
    import concourse.mybir as mybir
    import concourse.tile as tile
    from concourse import bacc

    f32 = mybir.dt.float32
    bf16 = mybir.dt.bfloat16
    Alu = mybir.AluOpType
    Act = mybir.ActivationFunctionType

    nc = bacc.Bacc(
        "TRN2",
        target_bir_lowering=False,
        debug=False,
        enable_asserts=True,
        num_devices=NCORES,
    )

    # ---- kernel I/O (per-core shapes; host shards the full problem) ----
    qT = nc.dram_tensor("qT", [C, N], bf16, kind="ExternalInput").ap()
    kT = nc.dram_tensor("kT", [C, N], bf16, kind="ExternalInput").ap()
    vT = nc.dram_tensor("vT", [C, N], bf16, kind="ExternalInput").ap()
    wq = nc.dram_tensor("wq", [C, CSP], bf16, kind="ExternalInput").ap()
    wk = nc.dram_tensor("wk", [C, CSP], bf16, kind="ExternalInput").ap()
    wv = nc.dram_tensor("wv", [C, CS], bf16, kind="ExternalInput").ap()
    wo = nc.dram_tensor("wo", [C, C], bf16, kind="ExternalInput").ap()
    bq = nc.dram_tensor("bq", [CS], f32, kind="ExternalInput").ap()
    bk = nc.dram_tensor("bk", [CS], f32, kind="ExternalInput").ap()
    bv = nc.dram_tensor("bv", [CS], f32, kind="ExternalInput").ap()
    bo = nc.dram_tensor("bo", [C], f32, kind="ExternalInput").ap()
    gamma = nc.dram_tensor("gamma", [C], f32, kind="ExternalInput").ap()
    beta = nc.dram_tensor("beta", [C], f32, kind="ExternalInput").ap()
    qres = nc.dram_tensor("qres", [QS, C], f32, kind="ExternalInput").ap()
    gsel = nc.dram_tensor("gsel", [2], f32, kind="ExternalInput").ap()
    y = nc.dram_tensor("y", [QS, C], f32, kind="ExternalOutput").ap()

    CI = C // P          # 6 contraction chunks
    NJ = N // 512        # 4 n-chunks of 512
    NM = N // P          # 16 kv-chunks of 128
    VS = DH + 1          # 65: v columns + ones column (denominator row)

    with tile.TileContext(nc) as tc:
        const = tc.alloc_tile_pool(name="const", bufs=1)
        persist = tc.alloc_tile_pool(name="persist", bufs=1)
        rows = tc.alloc_tile_pool(name="rows", bufs=2)
        ppool = tc.alloc_tile_pool(name="ppool", bufs=3)
        small = tc.alloc_tile_pool(name="small", bufs=4)
        dram = tc.alloc_tile_pool(name="dram", bufs=1, space="DRAM")

        # ---- constants (K first: the first matmuls need wk chunk 0 + kT row 0) ----
        wk_sb = const.tile([P, CI, CSP], bf16, name="wk_sb")
        wq_sb = const.tile([P, CI, CSP], bf16, name="wq_sb")
        wv_sb = const.tile([P, CI, CS], bf16, name="wv_sb")
        for i in range(CI):
            nc.sync.dma_start(
                wk_sb[:, i, :], wk.rearrange("(o p) m -> p o m", p=P)[:, i, :]
            )
        bkA = const.tile([P, 1], f32, name="bkA")
        bkB = const.tile([DH, 1], f32, name="bkB")
        nc.sync.dma_start(bkA[:], bk[0:P][:, None])
        nc.sync.dma_start(bkB[:], bk[P:CS][:, None])
        wo_sb = const.tile([P, CI, C], bf16, name="wo_sb")

        bqA = const.tile([P, 1], f32, name="bqA")
        bqB = const.tile([DH, 1], f32, name="bqB")
        bv_b = const.tile([P, CS], f32, name="bv_b")
        bo_b = const.tile([P, C], f32, name="bo_b")
        gamma_b = const.tile([P, C], f32, name="gamma_b")
        beta_b = const.tile([P, C], f32, name="beta_b")
        qres_sb = const.tile([P, QS // P, C], f32, name="qres_sb")

        # ---- persistent activations ----
        # Per-head K, zero-padded to the full 128 contraction rows so score
        # matmuls run in 128x128 mode (the zero rows null out the other
        # head's q values in the shared rhs).
        kP = [persist.tile([P, N], bf16, name=f"kP{h}") for h in range(HPC)]
        qTa = persist.tile([P, N], bf16, name="qTa")    # heads 0,1 q dims
        qTb = persist.tile([P, N], bf16, name="qTb")    # head 2 q dims (hi half 0)
        vaug = persist.tile([P, NM, HPC * VS], bf16, name="vaug")
        nc.vector.memset(kP[0][64:128, :], 0.0)
        nc.vector.memset(kP[1][0:64, :], 0.0)
        nc.vector.memset(kP[2][64:128, :], 0.0)
        nc.vector.memset(qTb[64:128, :], 0.0)
        nc.vector.memset(
            vaug.rearrange("p m (h d) -> p m h d", d=VS)[:, :, :, DH : DH + 1], 1.0
        )
        gs = const.tile([1, 2], f32, name="gs")
        s0b = const.tile([P, 1], f32, name="s0b")
        s1b = const.tile([P, 1], f32, name="s1b")
        o_h = [persist.tile([DH, N], f32, name=f"o{h}") for h in range(HPC)]
        o_hb = [persist.tile([DH, N], bf16, name=f"ob{h}") for h in range(HPC)]
        oG01 = persist.tile([P, 4, QS], bf16, name="oG01")
        oG2 = persist.tile([P, 2, QS], bf16, name="oG2")
        r_dram = dram.tile([2, 1024], f32, name="r_dram")

        a2a1_in = dram.tile([2 * NJ, 2 * DH, QS], bf16, name="a2a1_in")
        a2a1_out = dram.tile([2 * NJ, 2 * DH, QS], bf16, name="a2a1_out")
        a2a2_in = dram.tile([2 * NJ, DH, QS], bf16, name="a2a2_in")
        a2a2_out = dram.tile([2 * NJ, DH, QS], bf16, name="a2a2_out")

        # ================= Stage A: projections =================
        with tc.tile_pool(name="ppA", bufs=8, space="PSUM") as ppA:
            # --- K ---  (A half: heads 0,1 -> kP0/kP1; B half: head 2 -> kP2)
            pk_a = [ppA.tile([P, 512], f32, tag="acc", name=f"pka{j}") for j in range(NJ)]
            pk_b = [ppA.tile([P, 512], f32, tag="acc", name=f"pkb{j}") for j in range(NJ)]
            for i in range(CI):
                k_row = rows.tile([P, N], bf16, tag="row", bufs=6, name="k_row")
                nc.sync.dma_start(k_row[:], kT[P * i : P * (i + 1), :])
                st = dict(start=(i == 0), stop=(i == CI - 1))
                for j in range(NJ):
                    s5 = slice(512 * j, 512 * (j + 1))
                    nc.tensor.matmul(pk_a[j][:], wk_sb[:, i, 0:P], k_row[:, s5], **st)
                for j in range(NJ):
                    s5 = slice(512 * j, 512 * (j + 1))
                    nc.tensor.matmul(pk_b[j][:], wk_sb[:, i, P:CSP], k_row[:, s5], **st)
            for j in range(NJ):
                s5 = slice(512 * j, 512 * (j + 1))
                nc.vector.tensor_tensor(
                    kP[0][0:64, s5], pk_a[j][0:64], bkA[0:64].to_broadcast((64, 512)),
                    Alu.add,
                )
                nc.vector.tensor_tensor(
                    kP[1][64:128, s5], pk_a[j][64:128],
                    bkA[64:128].to_broadcast((64, 512)), Alu.add,
                )
                nc.vector.tensor_tensor(
                    kP[2][0:64, s5], pk_b[j][0:64], bkB.to_broadcast((64, 512)),
                    Alu.add,
                )
            # --- Q ---
            for i in range(CI):
                nc.sync.dma_start(
                    wq_sb[:, i, :], wq.rearrange("(o p) m -> p o m", p=P)[:, i, :]
                )
            nc.sync.dma_start(bqA[:], bq[0:P][:, None])
            nc.sync.dma_start(bqB[:], bq[P:CS][:, None])
            pq_a = [ppA.tile([P, 512], f32, tag="acc", name=f"pqa{j}") for j in range(NJ)]
            pq_b = [ppA.tile([P, 512], f32, tag="acc", name=f"pqb{j}") for j in range(NJ)]
            for i in range(CI):
                q_row = rows.tile([P, N], bf16, tag="row", bufs=6, name="q_row")
                nc.sync.dma_start(q_row[:], qT[P * i : P * (i + 1), :])
                st = dict(start=(i == 0), stop=(i == CI - 1))
                for j in range(NJ):
                    s5 = slice(512 * j, 512 * (j + 1))
                    nc.tensor.matmul(pq_a[j][:], wq_sb[:, i, 0:P], q_row[:, s5], **st)
                for j in range(NJ):
                    s5 = slice(512 * j, 512 * (j + 1))
                    nc.tensor.matmul(pq_b[j][:], wq_sb[:, i, P:CSP], q_row[:, s5], **st)
            for j in range(NJ):
                s5 = slice(512 * j, 512 * (j + 1))
                nc.vector.tensor_tensor(
                    qTa[:, s5], pq_a[j][:], bqA.to_broadcast((P, 512)), Alu.add
                )
                nc.vector.tensor_tensor(
                    qTb[0:64, s5], pq_b[j][0:64], bqB.to_broadcast((64, 512)), Alu.add
                )
            # --- V (natural layout, accumulated per kv-chunk) ---
            nc.sync.dma_start(wv_sb[:], wv.rearrange("(o p) m -> p o m", p=P))
            nc.sync.dma_start(bv_b[0:1, :], bv[None, :])
            nc.gpsimd.partition_broadcast(bv_b[:], bv_b[0:1, :])
            for half in range(2):
                pv = [
                    ppA.tile([P, CS], f32, tag="acc", name=f"pv{half}_{m8}")
                    for m8 in range(8)
                ]
                for i in range(CI):
                    v_half = rows.tile([P, 1024], bf16, tag="vrow", bufs=8, name="v_half")
                    nc.sync.dma_start(
                        v_half[:], vT[P * i : P * (i + 1), 1024 * half : 1024 * (half + 1)]
                    )
                    st = dict(start=(i == 0), stop=(i == CI - 1))
                    for m8 in range(8):
                        nc.tensor.matmul(
                            pv[m8][:, 0:CS],
                            v_half[:, P * m8 : P * (m8 + 1)],
                            wv_sb[:, i, :],
                            **st,
                        )
                for m8 in range(8):
                    m = 8 * half + m8
                    dst = vaug.rearrange("p m (h d) -> p m h d", d=VS)[:, m, :, 0:DH]
                    nc.vector.tensor_tensor(
                        dst,
                        pv[m8][:, 0:CS].rearrange("p (h d) -> p h d", d=DH),
                        bv_b.rearrange("p (h d) -> p h d", d=DH),
                        Alu.add,
                    )

        # ================= Stage B: attention (software-pipelined) ========
        with (
            tc.tile_pool(name="ppS", bufs=2, space="PSUM") as ppS,
            tc.tile_pool(name="ppO", bufs=4, space="PSUM") as ppO,
        ):
            def qsrc(h):
                return qTa if h < 2 else qTb

            def evict_divide(h, qh, po, rb_via_dma):
                qbase = 1024 * qh
                l_sb = small.tile([1, 1024], f32, tag="lr", bufs=4, name="l_sb")
                r_sb = small.tile([1, 1024], f32, tag="lr", bufs=4, name="r_sb")
                for q2 in range(2):
                    s5 = slice(qbase + 512 * q2, qbase + 512 * (q2 + 1))
                    nc.vector.tensor_copy(o_h[h][:, s5], po[q2][0:DH])
                    nc.vector.tensor_copy(
                        l_sb[0:1, 512 * q2 : 512 * (q2 + 1)],
                        po[q2][DH : DH + 1],
                    )
                sq = slice(qbase, qbase + 1024)
                nc.vector.reciprocal_approx_fast(out=r_sb[:], in_=l_sb[:])
                rb = ppool.tile([DH, 1024], f32, tag="rb", bufs=2, name="rb")
                if rb_via_dma:
                    # collective #1 occupies the gpsimd queue here; bounce the
                    # row through DRAM and broadcast it with a stride-0 DMA
                    nc.sync.dma_start(r_dram[qh : qh + 1, :], r_sb[0:1, :])
                    nc.sync.dma_start(
                        rb[:], r_dram[qh : qh + 1, :].to_broadcast((DH, 1024))
                    )
                else:
                    nc.gpsimd.partition_broadcast(rb[:], r_sb[0:1, :])
                nc.vector.tensor_tensor(
                    o_hb[h][:, sq], o_h[h][:, sq], rb[:], Alu.mult
                )

            def attend(h, qh, rb_via_dma):
                """Full 128x128-mode attention for head h over q-block qh."""
                qbase = 1024 * qh
                q_t = qsrc(h)
                po = [
                    ppO.tile([P, 512], f32, tag="o", name=f"po{qh}_{h}_{q2}")
                    for q2 in range(2)
                ]
                pts = [None] * NM
                for m in range(NM):
                    ps = ppS.tile([P, 1024], f32, tag="s", name=f"ps{qh}_{h}_{m}")
                    for q2 in range(2):
                        nc.tensor.matmul(
                            ps[:, 512 * q2 : 512 * (q2 + 1)],
                            kP[h][:, P * m : P * (m + 1)],
                            q_t[:, qbase + 512 * q2 : qbase + 512 * (q2 + 1)],
                            start=True,
                            stop=True,
                        )
                    pt = ppool.tile([P, 1024], bf16, tag="p", bufs=5, name="pt")
                    nc.scalar.activation(pt[:], ps[:], Act.Exp, scale=SCALE)
                    pts[m] = pt
                    if m >= 1:
                        for q2 in range(2):
                            nc.tensor.matmul(
                                po[q2][0:VS],
                                vaug[:, m - 1, VS * h : VS * (h + 1)],
                                pts[m - 1][:, 512 * q2 : 512 * (q2 + 1)],
                                start=(m - 1 == 0),
                                stop=False,
                            )
                        pts[m - 1] = None
                for q2 in range(2):
                    nc.tensor.matmul(
                        po[q2][0:VS],
                        vaug[:, NM - 1, VS * h : VS * (h + 1)],
                        pts[NM - 1][:, 512 * q2 : 512 * (q2 + 1)],
                        start=False,
                        stop=True,
                    )
                evict_divide(h, qh, po, rb_via_dma)

            # --- heads 0,1 over both q-blocks ---
            for qh in range(2):
                for h in (0, 1):
                    attend(h, qh, rb_via_dma=False)

            # stage heads 0,1 output and launch collective #1 (overlaps head 2)
            for r in range(NJ):
                for h in (0, 1):
                    for g2 in range(2):
                        nc.sync.dma_start(
                            a2a1_in[NJ * g2 + r, DH * h : DH * (h + 1), :],
                            o_hb[h][:, QS * r : QS * (r + 1)],
                        )
            nc.gpsimd.collective_compute(
                "AllToAll",
                Alu.bypass,
                replica_groups=[list(range(NCORES))],
                ins=[a2a1_in.opt()],
                outs=[a2a1_out.opt()],
            )
            nc.sync.dma_start(gs[:], gsel[None, :])
            nc.gpsimd.partition_broadcast(s0b[:], gs[0:1, 0:1])
            nc.gpsimd.partition_broadcast(s1b[:], gs[0:1, 1:2])

            # --- head 2 over both q-blocks (collective #1 in flight) ---
            for qh in range(2):
                attend(2, qh, rb_via_dma=True)

        # deferred tail-only constant loads (emitted late to keep the
        # startup DMA queue clear for stage A input rows)
        nc.sync.dma_start(wo_sb[:], wo.rearrange("(o p) m -> p o m", p=P))
        nc.sync.dma_start(bo_b[0:1, :], bo[None, :])
        nc.sync.dma_start(gamma_b[0:1, :], gamma[None, :])
        nc.sync.dma_start(beta_b[0:1, :], beta[None, :])
        nc.gpsimd.partition_broadcast(bo_b[:], bo_b[0:1, :])
        nc.gpsimd.partition_broadcast(gamma_b[:], gamma_b[0:1, :])
        nc.gpsimd.partition_broadcast(beta_b[:], beta_b[0:1, :])
        nc.sync.dma_start(qres_sb[:], qres.rearrange("(t p) c -> p t c", p=P))
        nc.vector.tensor_tensor(
            qres_sb[:],
            qres_sb[:],
            bo_b[:, None, :].to_broadcast((P, QS // P, C)),
            Alu.add,
        )

        # ====== Stage C: head-2 AllToAll + gathers ========================
        for r in range(NJ):
            for g2 in range(2):
                nc.sync.dma_start(
                    a2a2_in[NJ * g2 + r, :, :],
                    o_hb[2][:, QS * r : QS * (r + 1)],
                )
        nc.gpsimd.collective_compute(
            "AllToAll",
            Alu.bypass,
            replica_groups=[list(range(NCORES))],
            ins=[a2a2_in.opt()],
            outs=[a2a2_out.opt()],
        )

        # gather #1 (heads 0,1 of each source core) — runs during collective #2
        nc.sync.dma_start(
            oG01[:],
            a2a1_out[0:NJ, :, :].rearrange("r s w -> s r w"),
        )
        oGt1 = rows.tile([P, 4, QS], bf16, tag="row", bufs=6, name="oGt1")
        nc.sync.dma_start(
            oGt1[:],
            a2a1_out[NJ : 2 * NJ, :, :].rearrange("r s w -> s r w"),
        )
        nc.vector.tensor_scalar(oG01[:], oG01[:], s0b[:], None, Alu.mult)
        nc.vector.tensor_scalar(oGt1[:], oGt1[:], s1b[:], None, Alu.mult)
        nc.vector.tensor_tensor(oG01[:], oG01[:], oGt1[:], Alu.add)

        # gather #2 (head 2 of each source core)
        for r2 in range(2):
            nc.sync.dma_start(
                oG2.rearrange("(r2 s) o w -> r2 s o w", s=DH)[r2],
                a2a2_out[0:NJ, :, :].rearrange("(o r2) s w -> r2 s o w", r2=2)[r2],
            )
        oGt2 = rows.tile([P, 2, QS], bf16, tag="vrow", bufs=8, name="oGt2")
        for r2 in range(2):
            nc.sync.dma_start(
                oGt2[:].rearrange("(r2 s) o w -> r2 s o w", s=DH)[r2],
                a2a2_out[NJ : 2 * NJ, :, :].rearrange("(o r2) s w -> r2 s o w", r2=2)[r2],
            )
        nc.vector.tensor_scalar(oG2[:], oG2[:], s0b[:], None, Alu.mult)
        nc.vector.tensor_scalar(oGt2[:], oGt2[:], s1b[:], None, Alu.mult)
        nc.vector.tensor_tensor(oG2[:], oG2[:], oGt2[:], Alu.add)

        # ======= Stage D: full Wo (token-major) + residual + LayerNorm ====
        # ci 0..3 (heads 0,1 data) only needs gather #1 -> overlaps collective
        # #2; ci 4,5 (head 2) closes each accumulation group afterwards.
        with tc.tile_pool(name="ppD", bufs=4, space="PSUM") as ppD:
            px_t = []
            for qt in range(QS // P):
                px = ppD.tile([P, C], f32, tag="d", name=f"px{qt}")
                px_t.append(px)
                for ci in range(4):
                    st = dict(start=(ci == 0), stop=False)
                    nc.tensor.matmul(
                        px[:, 0:512],
                        oG01[:, ci, P * qt : P * (qt + 1)],
                        wo_sb[:, ci, 0:512],
                        **st,
                    )
                    nc.tensor.matmul(
                        px[:, 512:C],
                        oG01[:, ci, P * qt : P * (qt + 1)],
                        wo_sb[:, ci, 512:C],
                        **st,
                    )
            for qt in range(QS // P):
                px = px_t[qt]
                for ci in range(4, CI):
                    st = dict(start=False, stop=(ci == CI - 1))
                    nc.tensor.matmul(
                        px[:, 0:512],
                        oG2[:, ci - 4, P * qt : P * (qt + 1)],
                        wo_sb[:, ci, 0:512],
                        **st,
                    )
                    nc.tensor.matmul(
                        px[:, 512:C],
                        oG2[:, ci - 4, P * qt : P * (qt + 1)],
                        wo_sb[:, ci, 512:C],
                        **st,
                    )
                x1 = ppool.tile([P, C], f32, tag="x1", bufs=2, name="x1")
                nc.vector.tensor_tensor(x1[:], px[:], qres_sb[:, qt], Alu.add)
                mu = small.tile([P, 1], f32, tag="st", name="mu")
                sq = ppool.tile([P, C], f32, tag="sq", bufs=2, name="sq")
                sqs = small.tile([P, 1], f32, tag="st", name="sqs")
                var = small.tile([P, 1], f32, tag="st", name="var")
                rinv = small.tile([P, 1], f32, tag="st", name="rinv")
                rstd = small.tile([P, 1], f32, tag="st", name="rstd")
                nb = small.tile([P, 1], f32, tag="st", name="nb")
                nc.vector.reduce_sum(mu[:], x1[:], axis=mybir.AxisListType.X)
                nc.vector.tensor_scalar_mul(mu[:], mu[:], 1.0 / C)
                nc.scalar.activation(sq[:], x1[:], Act.Square, accum_out=sqs[:])
                nc.vector.tensor_scalar_mul(sqs[:], sqs[:], 1.0 / C)
                nc.vector.tensor_tensor(var[:], mu[:], mu[:], Alu.mult)
                nc.vector.tensor_tensor(var[:], sqs[:], var[:], Alu.subtract)
                nc.vector.tensor_scalar_add(var[:], var[:], EPS)
                nc.vector.reciprocal(rinv[:], var[:])
                nc.scalar.activation(rstd[:], rinv[:], Act.Sqrt)
                nc.vector.tensor_tensor(nb[:], mu[:], rstd[:], Alu.mult)
                nc.vector.tensor_scalar_mul(nb[:], nb[:], -1.0)
                nc.vector.tensor_scalar(
                    x1[:], x1[:], rstd[:], nb[:], Alu.mult, Alu.add
                )
                nc.vector.tensor_tensor(x1[:], x1[:], gamma_b[:], Alu.mult)
                nc.vector.tensor_tensor(x1[:], x1[:], beta_b[:], Alu.add)
                nc.sync.dma_start(
                    y.rearrange("(t p) c -> p t c", p=P)[:, qt], x1[:]
                )

        for pool in (dram, small, ppool, rows, persist, const):
            pool.release()

    nc.compile()
    return nc


def get_nc():
    if "nc" not in _NC_CACHE:
        _NC_CACHE["nc"] = _build_nc()
    return _NC_CACHE["nc"]


def make_in_maps(inputs):
    import ml_dtypes

    b16 = ml_dtypes.bfloat16
    q = np.asarray(inputs["query"], np.float32)
    k = np.asarray(inputs["key_in"], np.float32)
    v = np.asarray(inputs["value"], np.float32)
    Wq = np.asarray(inputs["Wq"], np.float32)
    Wk = np.asarray(inputs["Wk"], np.float32)
    Wv = np.asarray(inputs["Wv"], np.float32)
    Wo = np.asarray(inputs["Wo"], np.float32)
    bq = np.asarray(inputs["bq"], np.float32)
    bk = np.asarray(inputs["bk"], np.float32)
    bv = np.asarray(inputs["bv"], np.float32)
    bo = np.asarray(inputs["bo"], np.float32)
    gamma = np.asarray(inputs["gamma"], np.float32)
    beta = np.asarray(inputs["beta"], np.float32)

    def pad_w(w):  # [C, CS] -> [C, CSP] zero-padded
        out = np.zeros((C, CSP), np.float32)
        out[:, :CS] = w
        return out

    in_maps = []
    for c in range(NCORES):
        b, g = c // 4, c % 4
        cs = slice(CS * g, CS * (g + 1))
        in_maps.append(
            {
                "qT": np.ascontiguousarray(q[b].T).astype(b16),
                "kT": np.ascontiguousarray(k[b].T).astype(b16),
                "vT": np.ascontiguousarray(v[b].T).astype(b16),
                "wq": pad_w(Wq[:, cs]).astype(b16),
                "wk": pad_w(Wk[:, cs]).astype(b16),
                "wv": np.ascontiguousarray(Wv[:, cs]).astype(b16),
                "wo": Wo[WO_PERM, :].astype(b16),
                "bq": np.ascontiguousarray(bq[cs]),
                "bk": np.ascontiguousarray(bk[cs]),
                "bv": np.ascontiguousarray(bv[cs]),
                "bo": bo.copy(),
                "gamma": gamma.copy(),
                "beta": beta.copy(),
                "qres": np.ascontiguousarray(q[b, QS * g : QS * (g + 1)]),
                "gsel": np.array([1.0 - b, float(b)], np.float32),
            }
        )
    return in_maps


def _install_ntff_shim():
    """Provide antenv.axon_hooks if the image lacks it (needed for trace=True)."""
    try:
        import antenv.axon_hooks  # noqa: F401

        return
    except ImportError:
        pass
    import contextlib
    import ctypes
    import types

    so_path = "/opt/axon/libaxon_pjrt.so"
    state = {"hook": None}

    def set_axon_ntff_profile_hook(h):
        state["hook"] = h

    def get_axon_ntff_profile_hook():
        if state["hook"] is None:
            try:
                lib = ctypes.CDLL(so_path)
            except OSError:
                return None
            if not hasattr(lib, "axon_start_nrt_profile"):
                return None
            lib.axon_start_nrt_profile.argtypes = [
                ctypes.POINTER(ctypes.c_int64),
                ctypes.c_size_t,
            ]
            lib.axon_start_nrt_profile.restype = ctypes.c_int64
            lib.axon_stop_nrt_profile.argtypes = [ctypes.c_char_p]
            lib.axon_stop_nrt_profile.restype = ctypes.c_int64

            @contextlib.contextmanager
            def _hook(output_dir, device_ids):
                import jax

                jax.devices()
                if device_ids:
                    ids = (ctypes.c_int64 * len(device_ids))(*device_ids)
                    rc = lib.axon_start_nrt_profile(ids, len(device_ids))
                else:
                    rc = lib.axon_start_nrt_profile(None, 0)
                if rc != 0:
                    raise RuntimeError(f"axon_start_nrt_profile rc={rc}")
                try:
                    yield
                finally:
                    n = lib.axon_stop_nrt_profile(str(output_dir).encode())
                    print(f"profile: {n} file(s) written to {output_dir}")

            state["hook"] = _hook
        return state["hook"]

    mod = types.ModuleType("antenv.axon_hooks")
    mod.set_axon_ntff_profile_hook = set_axon_ntff_profile_hook
    mod.get_axon_ntff_profile_hook = get_axon_ntff_profile_hook
    import antenv

    antenv.axon_hooks = mod
    sys.modules["antenv.axon_hooks"] = mod


def run(inputs, trace=False, trace_cores=None):
    if trace:
        _install_ntff_shim()
    from concourse.bass_utils import run_bass_kernel_spmd

    nc = get_nc()
    in_maps = make_in_maps(inputs)
    res = run_bass_kernel_spmd(
        nc,
        in_maps,
        list(range(NCORES)),
        trace=trace,
        **({"trace_cores": trace_cores} if trace_cores is not None else {}),
    )
    out = np.empty((B, N, C), np.float32)
    for c in range(NCORES):
        b, g = c // 4, c % 4
        out[b, QS * g : QS * (g + 1)] = res.results[c]["y"]
    return out, res


def kernel(**inputs):
    out, _ = run(inputs, trace=False)
    return out


# revision 8
# speedup vs baseline: 1.2360x; 1.0507x over previous
"""Trainium2 Bass kernel for CrossAttention (B=2, N=2048, C=768, H=12).

Sharding: core c -> batch b=c//4, head-group g=c%4 (3 heads each).
Each core computes Q/K/V projections for its heads over the full sequence and
attention; per-head AllToAlls exchange outputs so each core then computes the
full output projection, residual and LayerNorm for its own 512-row q-shard.

v3: every matmul runs in the 128x128 tile mode (per-head K zero-padded to the
full contraction rows, head-2 projection weights zero-padded to 128 output
cols) so the PE never mode-switches and the HAM clock stays warm.  Attention
is head-major with one AllToAll per head: h0/h1 exchanges and their gather/
select run entirely under later attention; only the small h2 exchange tails.
A quarter of the softmax exps run on the vector engine via the Schraudolph
bit-trick (self-consistent numerator/denominator, ~5e-4 end-to-end) to keep
the scalar engine off the critical path.  LayerNorm's gamma/beta ops run on
gpsimd.

kernel(**inputs) takes the FULL inputs (setup_inputs() keys) and returns the
full [2, 2048, 768] output.
"""

import sys

for _p in ("/opt/trn_rl_repo",):
    if _p not in sys.path:
        sys.path.insert(0, _p)

import math

import numpy as np

B, N, C = 2, 2048, 768
H = 12
DH = 64
EPS = 1e-5
SCALE = DH ** (-0.5)  # 0.125

NCORES = 8
HPC = 3          # heads per core
CS = HPC * DH    # 192 output-feature slice per core
CSP = 256        # padded wq/wk col count (head2 half padded 64->128)
QS = N // 4      # 512 q rows per core
P = 128

# Schraudolph exp: exp(x) ~= bitcast_f32(int32(EA*x + EB)); scale folded in
EA = (2 ** 23 / math.log(2.0)) * SCALE
EB = float(127 * 2 ** 23 - 366393)
DVE_EXP_MS = (3, 7, 11, 15)   # m-steps whose exp runs on the vector engine

_NC_CACHE = {}

# Wo row permutation: gathered per-head AllToAll order is head-major
# [h0: g0|g1, g2|g3], [h1: ...], [h2: ...]; Wo rows must match.
import numpy as _np
WO_PERM = _np.concatenate(
    [
        _np.arange(192 * (2 * o + r2) + 64 * h, 192 * (2 * o + r2) + 64 * h + 64)
        for h in range(3)
        for o in range(2)
        for r2 in range(2)
    ]
)


def _build_nc():
    import concourse.bass as bass
    import concourse.mybir as mybir
    import concourse.tile as tile
    from concourse import bacc

    f32 = mybir.dt.float32
    i32 = mybir.dt.int32
    bf16 = mybir.dt.bfloat16
    Alu = mybir.AluOpType
    Act = mybir.ActivationFunctionType

    nc = bacc.Bacc(
        "TRN2",
        target_bir_lowering=False,
        debug=False,
        enable_asserts=True,
        num_devices=NCORES,
    )

    # ---- kernel I/O (per-core shapes; host shards the full problem) ----
    qT = nc.dram_tensor("qT", [C, N], bf16, kind="ExternalInput").ap()
    kT = nc.dram_tensor("kT", [C, N], bf16, kind="ExternalInput").ap()
    vT = nc.dram_tensor("vT", [C, N], bf16, kind="ExternalInput").ap()
    wq = nc.dram_tensor("wq", [C, CSP], bf16, kind="ExternalInput").ap()
    wk = nc.dram_tensor("wk", [C, CSP], bf16, kind="ExternalInput").ap()
    wv = nc.dram_tensor("wv", [C, CS], bf16, kind="ExternalInput").ap()
    wo = nc.dram_tensor("wo", [C, C], bf16, kind="ExternalInput").ap()
    bq = nc.dram_tensor("bq", [CS], f32, kind="ExternalInput").ap()
    bk = nc.dram_tensor("bk", [CS], f32, kind="ExternalInput").ap()
    bv = nc.dram_tensor("bv", [CS], f32, kind="ExternalInput").ap()
    bo = nc.dram_tensor("bo", [C], f32, kind="ExternalInput").ap()
    gamma = nc.dram_tensor("gamma", [C], f32, kind="ExternalInput").ap()
    beta = nc.dram_tensor("beta", [C], f32, kind="ExternalInput").ap()
    qres = nc.dram_tensor("qres", [QS, C], f32, kind="ExternalInput").ap()
    gsel = nc.dram_tensor("gsel", [2], f32, kind="ExternalInput").ap()
    y = nc.dram_tensor("y", [QS, C], f32, kind="ExternalOutput").ap()

    CI = C // P          # 6 contraction chunks
    NJ = N // 512        # 4 n-chunks of 512
    NM = N // P          # 16 kv-chunks of 128
    VS = DH + 1          # 65: v columns + ones column (denominator row)

    with tile.TileContext(nc) as tc:
        const = tc.alloc_tile_pool(name="const", bufs=1)
        persist = tc.alloc_tile_pool(name="persist", bufs=1)
        rows = tc.alloc_tile_pool(name="rows", bufs=2)
        ppool = tc.alloc_tile_pool(name="ppool", bufs=3)
        small = tc.alloc_tile_pool(name="small", bufs=4)
        dram = tc.alloc_tile_pool(name="dram", bufs=1, space="DRAM")

        # ---- constants (K first: the first matmuls need wk chunk 0 + kT row 0) ----
        wk_sb = const.tile([P, CI, CSP], bf16, name="wk_sb")
        wq_sb = const.tile([P, CI, CSP], bf16, name="wq_sb")
        wv_sb = const.tile([P, CI, CS], bf16, name="wv_sb")
        for i in range(CI):
            nc.sync.dma_start(
                wk_sb[:, i, :], wk.rearrange("(o p) m -> p o m", p=P)[:, i, :]
            )
        bkA = const.tile([P, 1], f32, name="bkA")
        bkB = const.tile([DH, 1], f32, name="bkB")
        nc.sync.dma_start(bkA[:], bk[0:P][:, None])
        nc.sync.dma_start(bkB[:], bk[P:CS][:, None])
        wo_sb = const.tile([P, CI, C], bf16, name="wo_sb")

        bqA = const.tile([P, 1], f32, name="bqA")
        bqB = const.tile([DH, 1], f32, name="bqB")
        bv_b = const.tile([P, CS], f32, name="bv_b")
        bo_b = const.tile([P, C], f32, name="bo_b")
        gamma_b = const.tile([P, C], f32, name="gamma_b")
        beta_b = const.tile([P, C], f32, name="beta_b")
        qres_sb = const.tile([P, QS // P, C], f32, name="qres_sb")

        # ---- persistent activations ----
        kP = [persist.tile([P, N], bf16, name=f"kP{h}") for h in range(HPC)]
        qTa = persist.tile([P, N], bf16, name="qTa")    # heads 0,1 q dims
        qTb = persist.tile([P, N], bf16, name="qTb")    # head 2 q dims (hi half 0)
        vaug = persist.tile([P, NM, HPC * VS], bf16, name="vaug")
        nc.vector.memset(kP[0][64:128, :], 0.0)
        nc.vector.memset(kP[1][0:64, :], 0.0)
        nc.vector.memset(kP[2][64:128, :], 0.0)
        nc.vector.memset(qTb[64:128, :], 0.0)
        nc.vector.memset(
            vaug.rearrange("p m (h d) -> p m h d", d=VS)[:, :, :, DH : DH + 1], 1.0
        )
        gs = const.tile([1, 2], f32, name="gs")
        s0b = const.tile([P, 1], f32, name="s0b")
        s1b = const.tile([P, 1], f32, name="s1b")
        o_h = [persist.tile([DH, N], bf16, name=f"o{h}") for h in range(HPC)]
        o_hb = [persist.tile([DH, N], bf16, name=f"ob{h}") for h in range(HPC)]
        oG = [persist.tile([P, 2, QS], bf16, name=f"oG{h}") for h in range(HPC)]
        r_dram = dram.tile([2 * HPC, 1024], f32, name="r_dram")

        a2a_in = [
            dram.tile([2 * NJ, DH, QS], bf16, name=f"a2a{h}_in") for h in range(HPC)
        ]
        a2a_out = [
            dram.tile([2 * NJ, DH, QS], bf16, name=f"a2a{h}_out") for h in range(HPC)
        ]

        # ================= Stage A: projections =================
        with tc.tile_pool(name="ppA", bufs=8, space="PSUM") as ppA:
            # --- K ---  (A half: heads 0,1 -> kP0/kP1; B half: head 2 -> kP2)
            pk_a = [ppA.tile([P, 512], f32, tag="acc", name=f"pka{j}") for j in range(NJ)]
            pk_b = [ppA.tile([P, 512], f32, tag="acc", name=f"pkb{j}") for j in range(NJ)]
            for i in range(CI):
                k_row = rows.tile([P, N], bf16, tag="row", bufs=6, name="k_row")
                nc.sync.dma_start(k_row[:], kT[P * i : P * (i + 1), :])
                st = dict(start=(i == 0), stop=(i == CI - 1))
                for j in range(NJ):
                    s5 = slice(512 * j, 512 * (j + 1))
                    nc.tensor.matmul(pk_a[j][:], wk_sb[:, i, 0:P], k_row[:, s5], **st)
                for j in range(NJ):
                    s5 = slice(512 * j, 512 * (j + 1))
                    nc.tensor.matmul(pk_b[j][:], wk_sb[:, i, P:CSP], k_row[:, s5], **st)
            for j in range(NJ):
                s5 = slice(512 * j, 512 * (j + 1))
                nc.vector.tensor_tensor(
                    kP[0][0:64, s5], pk_a[j][0:64], bkA[0:64].to_broadcast((64, 512)),
                    Alu.add,
                )
                nc.vector.tensor_tensor(
                    kP[1][64:128, s5], pk_a[j][64:128],
                    bkA[64:128].to_broadcast((64, 512)), Alu.add,
                )
                nc.vector.tensor_tensor(
                    kP[2][0:64, s5], pk_b[j][0:64], bkB.to_broadcast((64, 512)),
                    Alu.add,
                )
            # --- Q ---
            for i in range(CI):
                nc.sync.dma_start(
                    wq_sb[:, i, :], wq.rearrange("(o p) m -> p o m", p=P)[:, i, :]
                )
            nc.sync.dma_start(bqA[:], bq[0:P][:, None])
            nc.sync.dma_start(bqB[:], bq[P:CS][:, None])
            pq_a = [ppA.tile([P, 512], f32, tag="acc", name=f"pqa{j}") for j in range(NJ)]
            pq_b = [ppA.tile([P, 512], f32, tag="acc", name=f"pqb{j}") for j in range(NJ)]
            for i in range(CI):
                q_row = rows.tile([P, N], bf16, tag="row", bufs=6, name="q_row")
                nc.sync.dma_start(q_row[:], qT[P * i : P * (i + 1), :])
                st = dict(start=(i == 0), stop=(i == CI - 1))
                for j in range(NJ):
                    s5 = slice(512 * j, 512 * (j + 1))
                    nc.tensor.matmul(pq_a[j][:], wq_sb[:, i, 0:P], q_row[:, s5], **st)
                for j in range(NJ):
                    s5 = slice(512 * j, 512 * (j + 1))
                    nc.tensor.matmul(pq_b[j][:], wq_sb[:, i, P:CSP], q_row[:, s5], **st)
            for j in range(NJ):
                s5 = slice(512 * j, 512 * (j + 1))
                nc.vector.tensor_tensor(
                    qTa[:, s5], pq_a[j][:], bqA.to_broadcast((P, 512)), Alu.add
                )
                nc.vector.tensor_tensor(
                    qTb[0:64, s5], pq_b[j][0:64], bqB.to_broadcast((64, 512)), Alu.add
                )
            # --- V (natural layout, accumulated per kv-chunk) ---
            nc.sync.dma_start(wv_sb[:], wv.rearrange("(o p) m -> p o m", p=P))
            nc.sync.dma_start(bv_b[0:1, :], bv[None, :])
            nc.gpsimd.partition_broadcast(bv_b[:], bv_b[0:1, :])
            for half in range(2):
                pv = [
                    ppA.tile([P, CS], f32, tag="acc", name=f"pv{half}_{m8}")
                    for m8 in range(8)
                ]
                for i in range(CI):
                    v_half = rows.tile([P, 1024], bf16, tag="vrow", bufs=8, name="v_half")
                    nc.sync.dma_start(
                        v_half[:], vT[P * i : P * (i + 1), 1024 * half : 1024 * (half + 1)]
                    )
                    st = dict(start=(i == 0), stop=(i == CI - 1))
                    for m8 in range(8):
                        nc.tensor.matmul(
                            pv[m8][:, 0:CS],
                            v_half[:, P * m8 : P * (m8 + 1)],
                            wv_sb[:, i, :],
                            **st,
                        )
                for m8 in range(8):
                    m = 8 * half + m8
                    dst = vaug.rearrange("p m (h d) -> p m h d", d=VS)[:, m, :, 0:DH]
                    nc.vector.tensor_tensor(
                        dst,
                        pv[m8][:, 0:CS].rearrange("p (h d) -> p h d", d=DH),
                        bv_b.rearrange("p (h d) -> p h d", d=DH),
                        Alu.add,
                    )

        # ================= Stage B: attention (head-major, pipelined) ======
        with (
            tc.tile_pool(name="ppS", bufs=3, space="PSUM") as ppS,
            tc.tile_pool(name="ppO", bufs=2, space="PSUM") as ppO,
        ):
            def qsrc(h):
                return qTa if h < 2 else qTb

            def evict_divide(h, qh, po):
                qbase = 1024 * qh
                l_sb = small.tile([1, 1024], f32, tag="lr", bufs=3, name="l_sb")
                r_sb = small.tile([1, 1024], f32, tag="lr", bufs=3, name="r_sb")
                for q2 in range(2):
                    s5 = slice(qbase + 512 * q2, qbase + 512 * (q2 + 1))
                    nc.vector.tensor_copy(o_h[h][:, s5], po[q2][0:DH])
                    nc.vector.tensor_copy(
                        l_sb[0:1, 512 * q2 : 512 * (q2 + 1)],
                        po[q2][DH : DH + 1],
                    )
                sq = slice(qbase, qbase + 1024)
                nc.vector.reciprocal_approx_fast(out=r_sb[:], in_=l_sb[:])
                rb = ppool.tile([DH, 1024], f32, tag="rb", bufs=2, name="rb")
                # bounce the reciprocal row through DRAM and partition-
                # broadcast it with a stride-0 DMA (gpsimd runs collectives)
                rd = 2 * h + qh
                nc.sync.dma_start(r_dram[rd : rd + 1, :], r_sb[0:1, :])
                nc.sync.dma_start(
                    rb[:], r_dram[rd : rd + 1, :].to_broadcast((DH, 1024))
                )
                nc.vector.tensor_tensor(
                    o_hb[h][:, sq], o_h[h][:, sq], rb[:], Alu.mult
                )

            def emit_exp(h, m, ps, pt):
                if m in DVE_EXP_MS:
                    # Schraudolph bit-trick exp on the vector engine
                    it = small.tile([P, 1024], i32, tag="i32", bufs=2, name="it")
                    nc.vector.tensor_scalar(it[:], ps[:], EA, EB, Alu.mult, Alu.add)
                    nc.vector.tensor_copy(pt[:], it[:].bitcast(f32))
                else:
                    nc.scalar.activation(pt[:], ps[:], Act.Exp, scale=SCALE)

            def attend(h, qh):
                """Full 128x128-mode attention for head h over q-block qh."""
                qbase = 1024 * qh
                q_t = qsrc(h)
                po = [
                    ppO.tile([P, 512], f32, tag="o", name=f"po{qh}_{h}_{q2}")
                    for q2 in range(2)
                ]
                pts = {}
                pend = []

                def emit_av(mm):
                    for q2 in range(2):
                        nc.tensor.matmul(
                            po[q2][0:VS],
                            vaug[:, mm, VS * h : VS * (h + 1)],
                            pts[mm][:, 512 * q2 : 512 * (q2 + 1)],
                            start=(mm == 0),
                            stop=(mm == NM - 1),
                        )
                    del pts[mm]
                for m in range(NM):
                    ps = ppS.tile([P, 1024], f32, tag="s", name=f"ps{qh}_{h}_{m}")
                    for q2 in range(2):
                        nc.tensor.matmul(
                            ps[:, 512 * q2 : 512 * (q2 + 1)],
                            kP[h][:, P * m : P * (m + 1)],
                            q_t[:, qbase + 512 * q2 : qbase + 512 * (q2 + 1)],
                            start=True,
                            stop=True,
                        )
                    pt = ppool.tile([P, 1024], bf16, tag="p", bufs=5, name="pt")
                    emit_exp(h, m, ps, pt)
                    pts[m] = pt
                    pend.append(m)
                    # slow-engine exps get an extra pipeline step of lead time
                    lag = 2 if pend[0] in DVE_EXP_MS else 1
                    while pend and pend[0] <= m - lag:
                        emit_av(pend.pop(0))
                        if pend:
                            lag = 2 if pend[0] in DVE_EXP_MS else 1
                for mm in list(pend):
                    emit_av(mm)
                evict_divide(h, qh, po)

            def stage_a2a(h):
                for r in range(NJ):
                    for g2 in range(2):
                        nc.sync.dma_start(
                            a2a_in[h][NJ * g2 + r, :, :],
                            o_hb[h][:, QS * r : QS * (r + 1)],
                        )
                nc.gpsimd.collective_compute(
                    "AllToAll",
                    mybir.AluOpType.bypass,
                    replica_groups=[list(range(NCORES))],
                    ins=[a2a_in[h].opt()],
                    outs=[a2a_out[h].opt()],
                )

            def gather_select(h):
                # two batch-halves of the exchange, select own batch via gsel
                for r2 in range(2):
                    nc.sync.dma_start(
                        oG[h].rearrange("(r2 s) o w -> r2 s o w", s=DH)[r2],
                        a2a_out[h][0:NJ, :, :].rearrange(
                            "(o r2) s w -> r2 s o w", r2=2
                        )[r2],
                    )
                oGt = rows.tile([P, 2, QS], bf16, tag="vrow", bufs=8, name="oGt")
                for r2 in range(2):
                    nc.sync.dma_start(
                        oGt[:].rearrange("(r2 s) o w -> r2 s o w", s=DH)[r2],
                        a2a_out[h][NJ : 2 * NJ, :, :].rearrange(
                            "(o r2) s w -> r2 s o w", r2=2
                        )[r2],
                    )
                nc.vector.tensor_scalar(oG[h][:], oG[h][:], s0b[:], None, Alu.mult)
                nc.vector.tensor_scalar(oGt[:], oGt[:], s1b[:], None, Alu.mult)
                nc.vector.tensor_tensor(oG[h][:], oG[h][:], oGt[:], Alu.add)

            # ---- head 0 ----
            attend(0, 0)
            attend(0, 1)
            stage_a2a(0)
            nc.sync.dma_start(gs[:], gsel[None, :])
            nc.gpsimd.partition_broadcast(s0b[:], gs[0:1, 0:1])
            nc.gpsimd.partition_broadcast(s1b[:], gs[0:1, 1:2])
            # ---- head 1 (h0 exchange in flight) ----
            attend(1, 0)
            attend(1, 1)
            stage_a2a(1)
            gather_select(0)
            # ---- head 2 (h1 exchange in flight) ----
            attend(2, 0)
            gather_select(1)
            attend(2, 1)
            stage_a2a(2)

        # deferred tail-only constant loads
        nc.sync.dma_start(wo_sb[:], wo.rearrange("(o p) m -> p o m", p=P))
        nc.sync.dma_start(bo_b[0:1, :], bo[None, :])
        nc.sync.dma_start(gamma_b[0:1, :], gamma[None, :])
        nc.sync.dma_start(beta_b[0:1, :], beta[None, :])
        nc.gpsimd.partition_broadcast(bo_b[:], bo_b[0:1, :])
        nc.gpsimd.partition_broadcast(gamma_b[:], gamma_b[0:1, :])
        nc.gpsimd.partition_broadcast(beta_b[:], beta_b[0:1, :])
        nc.sync.dma_start(qres_sb[:], qres.rearrange("(t p) c -> p t c", p=P))
        nc.vector.tensor_tensor(
            qres_sb[:],
            qres_sb[:],
            bo_b[:, None, :].to_broadcast((P, QS // P, C)),
            Alu.add,
        )
        gather_select(2)

        # ======= Stage D: full Wo (token-major) + residual + LayerNorm ====
        # ci 0..3 (heads 0,1) only needs gathers already done under attention;
        # ci 4,5 (head 2) closes each accumulation group after exchange #3.
        with tc.tile_pool(name="ppD", bufs=4, space="PSUM") as ppD:
            px_t = []
            for qt in range(QS // P):
                px = ppD.tile([P, C], f32, tag="d", name=f"px{qt}")
                px_t.append(px)
                for ci in range(4):
                    st = dict(start=(ci == 0), stop=False)
                    lhsT = oG[ci // 2][:, ci % 2, P * qt : P * (qt + 1)]
                    nc.tensor.matmul(px[:, 0:512], lhsT, wo_sb[:, ci, 0:512], **st)
                    nc.tensor.matmul(px[:, 512:C], lhsT, wo_sb[:, ci, 512:C], **st)
            for qt in range(QS // P):
                px = px_t[qt]
                for ci in range(4, CI):
                    st = dict(start=False, stop=(ci == CI - 1))
                    lhsT = oG[2][:, ci - 4, P * qt : P * (qt + 1)]
                    nc.tensor.matmul(px[:, 0:512], lhsT, wo_sb[:, ci, 0:512], **st)
                    nc.tensor.matmul(px[:, 512:C], lhsT, wo_sb[:, ci, 512:C], **st)
                x1 = ppool.tile([P, C], f32, tag="x1", bufs=2, name="x1")
                nc.vector.tensor_tensor(x1[:], px[:], qres_sb[:, qt], Alu.add)
                mu = small.tile([P, 1], f32, tag="st", name="mu")
                sq = ppool.tile([P, C], f32, tag="sq", bufs=2, name="sq")
                sqs = small.tile([P, 1], f32, tag="st", name="sqs")
                var = small.tile([P, 1], f32, tag="st", name="var")
                rinv = small.tile([P, 1], f32, tag="st", name="rinv")
                rstd = small.tile([P, 1], f32, tag="st", name="rstd")
                nb = small.tile([P, 1], f32, tag="st", name="nb")
                nc.vector.reduce_sum(mu[:], x1[:], axis=mybir.AxisListType.X)
                nc.vector.tensor_scalar_mul(mu[:], mu[:], 1.0 / C)
                nc.scalar.activation(sq[:], x1[:], Act.Square, accum_out=sqs[:])
                nc.vector.tensor_scalar_mul(sqs[:], sqs[:], 1.0 / C)
                nc.vector.tensor_tensor(var[:], mu[:], mu[:], Alu.mult)
                nc.vector.tensor_tensor(var[:], sqs[:], var[:], Alu.subtract)
                nc.vector.tensor_scalar_add(var[:], var[:], EPS)
                nc.vector.reciprocal(rinv[:], var[:])
                nc.scalar.activation(rstd[:], rinv[:], Act.Sqrt)
                nc.vector.tensor_tensor(nb[:], mu[:], rstd[:], Alu.mult)
                nc.vector.tensor_scalar_mul(nb[:], nb[:], -1.0)
                nc.vector.tensor_scalar(
                    x1[:], x1[:], rstd[:], nb[:], Alu.mult, Alu.add
                )
                # gamma/beta on gpsimd: the vector engine is the tail
                # bottleneck and gpsimd is idle once collectives are done
                nc.gpsimd.tensor_tensor(x1[:], x1[:], gamma_b[:], Alu.mult)
                nc.gpsimd.tensor_tensor(x1[:], x1[:], beta_b[:], Alu.add)
                nc.sync.dma_start(
                    y.rearrange("(t p) c -> p t c", p=P)[:, qt], x1[:]
                )

        for pool in (dram, small, ppool, rows, persist, const):
            pool.release()

    nc.compile()
    return nc


def get_nc():
    if "nc" not in _NC_CACHE:
        _NC_CACHE["nc"] = _build_nc()
    return _NC_CACHE["nc"]


def make_in_maps(inputs):
    import ml_dtypes

    b16 = ml_dtypes.bfloat16
    q = np.asarray(inputs["query"], np.float32)
    k = np.asarray(inputs["key_in"], np.float32)
    v = np.asarray(inputs["value"], np.float32)
    Wq = np.asarray(inputs["Wq"], np.float32)
    Wk = np.asarray(inputs["Wk"], np.float32)
    Wv = np.asarray(inputs["Wv"], np.float32)
    Wo = np.asarray(inputs["Wo"], np.float32)
    bq = np.asarray(inputs["bq"], np.float32)
    bk = np.asarray(inputs["bk"], np.float32)
    bv = np.asarray(inputs["bv"], np.float32)
    bo = np.asarray(inputs["bo"], np.float32)
    gamma = np.asarray(inputs["gamma"], np.float32)
    beta = np.asarray(inputs["beta"], np.float32)

    def pad_w(w):  # [C, CS] -> [C, CSP] zero-padded
        out = np.zeros((C, CSP), np.float32)
        out[:, :CS] = w
        return out

    in_maps = []
    for c in range(NCORES):
        b, g = c // 4, c % 4
        cs = slice(CS * g, CS * (g + 1))
        in_maps.append(
            {
                "qT": np.ascontiguousarray(q[b].T).astype(b16),
                "kT": np.ascontiguousarray(k[b].T).astype(b16),
                "vT": np.ascontiguousarray(v[b].T).astype(b16),
                "wq": pad_w(Wq[:, cs]).astype(b16),
                "wk": pad_w(Wk[:, cs]).astype(b16),
                "wv": np.ascontiguousarray(Wv[:, cs]).astype(b16),
                "wo": Wo[WO_PERM, :].astype(b16),
                "bq": np.ascontiguousarray(bq[cs]),
                "bk": np.ascontiguousarray(bk[cs]),
                "bv": np.ascontiguousarray(bv[cs]),
                "bo": bo.copy(),
                "gamma": gamma.copy(),
                "beta": beta.copy(),
                "qres": np.ascontiguousarray(q[b, QS * g : QS * (g + 1)]),
                "gsel": np.array([1.0 - b, float(b)], np.float32),
            }
        )
    return in_maps


def _install_ntff_shim():
    """Provide antenv.axon_hooks if the image lacks it (needed for trace=True)."""
    try:
        import antenv.axon_hooks  # noqa: F401

        return
    except ImportError:
        pass
    import contextlib
    import ctypes
    import types

    so_path = "/opt/axon/libaxon_pjrt.so"
    state = {"hook": None}

    def set_axon_ntff_profile_hook(h):
        state["hook"] = h

    def get_axon_ntff_profile_hook():
        if state["hook"] is None:
            try:
                lib = ctypes.CDLL(so_path)
            except OSError:
                return None
            if not hasattr(lib, "axon_start_nrt_profile"):
                return None
            lib.axon_start_nrt_profile.argtypes = [
                ctypes.POINTER(ctypes.c_int64),
                ctypes.c_size_t,
            ]
            lib.axon_start_nrt_profile.restype = ctypes.c_int64
            lib.axon_stop_nrt_profile.argtypes = [ctypes.c_char_p]
            lib.axon_stop_nrt_profile.restype = ctypes.c_int64

            @contextlib.contextmanager
            def _hook(output_dir, device_ids):
                import jax

                jax.devices()
                if device_ids:
                    ids = (ctypes.c_int64 * len(device_ids))(*device_ids)
                    rc = lib.axon_start_nrt_profile(ids, len(device_ids))
                else:
                    rc = lib.axon_start_nrt_profile(None, 0)
                if rc != 0:
                    raise RuntimeError(f"axon_start_nrt_profile rc={rc}")
                try:
                    yield
                finally:
                    n = lib.axon_stop_nrt_profile(str(output_dir).encode())
                    print(f"profile: {n} file(s) written to {output_dir}")

            state["hook"] = _hook
        return state["hook"]

    mod = types.ModuleType("antenv.axon_hooks")
    mod.set_axon_ntff_profile_hook = set_axon_ntff_profile_hook
    mod.get_axon_ntff_profile_hook = get_axon_ntff_profile_hook
    import antenv

    antenv.axon_hooks = mod
    sys.modules["antenv.axon_hooks"] = mod


def run(inputs, trace=False, trace_cores=None):
    if trace:
        _install_ntff_shim()
    from concourse.bass_utils import run_bass_kernel_spmd

    nc = get_nc()
    in_maps = make_in_maps(inputs)
    res = run_bass_kernel_spmd(
        nc,
        in_maps,
        list(range(NCORES)),
        trace=trace,
        **({"trace_cores": trace_cores} if trace_cores is not None else {}),
    )
    out = np.empty((B, N, C), np.float32)
    for c in range(NCORES):
        b, g = c // 4, c % 4
        out[b, QS * g : QS * (g + 1)] = res.results[c]["y"]
    return out, res


def kernel(**inputs):
    out, _ = run(inputs, trace=False)
    return out
